# revision 11
# baseline (speedup 1.0000x reference)
"""Trainium2 Bass kernel for nn_Attention_47768626266365.

Dense transformer block: ChanLayerNorm -> 1x1 conv qkv -> depthwise 3x3 convs
-> 8-head attention with relative-position bias -> 1x1 conv out.

Sharding: data-parallel over batch, 2 images per core across 8 cores.

v3 — single fine-grained weave tuned for PE density (HAM stays warm):
  * LayerNorm folded into the qkv projection (rstd fused into the PSUM
    evacuation, mean rides the accumulation as a K=1 matmul).
  * depthwise 3x3 split symmetrically for both images:
      6 taps on PE (diag-matmul accumulated into the reused qkv PSUM),
      2 on scalar (per-partition-scale Copy), 1 on DVE (fused with the
      PSUM evacuation via scalar_tensor_tensor), scalar pair merged on
      GPSIMD, final merge on DVE.
  * the whole kernel is ONE interleaved stream: stage-1 oc-steps of both
    images are paced by linear ramps underneath the 128-step attention
    stream, so the PE queue always has independent matmuls between the
    sim->exp->mult->AV dependency chains.
  * AV lags its sim by one slot; sim PSUM double-buffered; av single
    buffered; qkv/out-proj PSUM single buffered: 2+4+2 = 8 banks exactly.
  * f16 denominator broadcasts, per-pair reciprocal, DVE-squared LN stats
    (keeps the scalar engine in the natural_log_exp table: 1 table load).
"""

import os
import sys

sys.path.insert(0, "/opt/trn_rl_repo")

import numpy as np
from contextlib import ExitStack

import concourse.bass as bass
import concourse.bacc as bacc
import concourse.mybir as mybir
import concourse.tile as tile
from concourse.bass_utils import run_bass_kernel_spmd


F32 = mybir.dt.float32
F16 = mybir.dt.float16
AF = mybir.ActivationFunctionType
OP = mybir.AluOpType

# ---- problem constants (hardcoded per contract) ----
B, C, S = 16, 512, 32
TOK = S * S                     # 1024 tokens
HEADS, D = 8, 64
INNER = HEADS * D               # 512
O3 = 3 * INNER                  # 1536 qkv channels
NCORES = 8
IPC = B // NCORES               # images per core = 2
P = 128
PW = S + 2                      # padded row width 34
PTOK = PW * PW + 2              # 1156 + slack
EPS = 1e-5
SCALE = D ** -0.5
NOC = O3 // P                   # 12 qkv channel chunks
NCC = C // P                    # 4 input channel chunks
NJC = TOK // P                  # 8 token chunks

# canonical tap order for host-side dwW packing
TAPS = [(dx, dy) for dx in (-1, 0, 1) for dy in (-1, 0, 1)]
PE_TAPS = [(-1, 0), (0, 0), (1, 0), (-1, -1), (-1, 1), (1, -1)]
SC_TAPS = [(0, -1), (0, 1)]
DV_TAP = (1, 1)
OC_ORDER = [0, 4, 8, 1, 5, 9, 2, 6, 10, 3, 7, 11]


def _tidx(tap):
    return TAPS.index(tap)


def _pad_view(t, off, rows):
    """(128, rows, 32) view into padded (128, PTOK) tile at element offset."""
    return t[:, off: off + rows * PW].rearrange("p (x y) -> p x y", y=PW)[:, :, :S]


def _tap_off(dx, dy):
    return (1 + dx) * PW + (1 + dy)


def build_nc():
    nc = bacc.Bacc("TRN2", target_bir_lowering=False, debug=False)

    x_d = nc.dram_tensor("x", (IPC, C, TOK), F32, kind="ExternalInput")
    wqkvT_d = nc.dram_tensor("wqkvT", (P, NCC, O3), F16, kind="ExternalInput")
    negwsum_d = nc.dram_tensor("negwsum", (1, O3), F16, kind="ExternalInput")
    woutT_d = nc.dram_tensor("woutT", (P, NCC, INNER), F16, kind="ExternalInput")
    dwW_d = nc.dram_tensor("dwW", (P, NOC, 9), F32, kind="ExternalInput")
    dwdiag_d = nc.dram_tensor(
        "dwdiag", (P, NOC, len(PE_TAPS), P), F16, kind="ExternalInput")
    ebt_d = nc.dram_tensor("ebt", (HEADS, NJC, P, TOK), F16, kind="ExternalInput")
    selpair_d = nc.dram_tensor("selpair", (2, P), F16, kind="ExternalInput")
    out_d = nc.dram_tensor("out", (IPC, C, TOK), F32, kind="ExternalOutput")

    with tile.TileContext(nc) as tc, ExitStack() as ctx:
        const = ctx.enter_context(tc.tile_pool(name="const", bufs=1))
        persist = ctx.enter_context(tc.tile_pool(name="persist", bufs=1))
        qp = ctx.enter_context(tc.tile_pool(name="qp", bufs=3))
        accp = ctx.enter_context(tc.tile_pool(name="accp", bufs=7))
        ttp = ctx.enter_context(tc.tile_pool(name="ttp", bufs=4))
        ep = ctx.enter_context(tc.tile_pool(name="ep", bufs=5))
        Ep = ctx.enter_context(tc.tile_pool(name="Ep", bufs=3))
        ofp = ctx.enter_context(tc.tile_pool(name="ofp", bufs=2))
        small = ctx.enter_context(tc.tile_pool(name="small", bufs=2))
        rcp = ctx.enter_context(tc.tile_pool(name="rcp", bufs=1))
        vtp = ctx.enter_context(tc.tile_pool(name="vtp", bufs=2))

        # PSUM: prologue pool (LN stats, closed before attention) + single
        # qkv/out-proj accumulator; sim/av pools open after the prologue.
        psqP = ctx.enter_context(tc.tile_pool(name="psqP", bufs=1, space="PSUM"))
        pro_ctx = ExitStack()
        proP = pro_ctx.enter_context(
            tc.tile_pool(name="proP", bufs=3, space="PSUM"))

        # ---------- constants ----------
        wqkvT = const.tile([P, NCC, O3], F16, tag="wqkvT")
        nc.sync.dma_start(wqkvT[:], wqkvT_d[:])
        negwsum = const.tile([1, O3], F16, tag="negwsum")
        nc.sync.dma_start(negwsum[:], negwsum_d[:])
        woutT = const.tile([P, NCC, INNER], F16, tag="woutT")
        nc.sync.dma_start(woutT[:], woutT_d[:])
        dwW = const.tile([P, NOC, 9], F32, tag="dwW")
        nc.sync.dma_start(dwW[:], dwW_d[:])
        dwdiag = const.tile([P, NOC, len(PE_TAPS), P], F16, tag="dwdiag")
        nc.sync.dma_start(dwdiag[:], dwdiag_d[:])
        selA = const.tile([1, P], F16, tag="selA")
        nc.sync.dma_start(selA[:], selpair_d[0:1, :])
        selB = const.tile([1, P], F16, tag="selB")
        nc.sync.dma_start(selB[:], selpair_d[1:2, :])
        ones128 = const.tile([P, 1], F16, tag="ones128")
        nc.gpsimd.memset(ones128[:], 1.0)
        onesrow = const.tile([1, P], F16, tag="onesrow")
        nc.gpsimd.memset(onesrow[:], 1.0)
        epsc = const.tile([1, 1], F32, tag="epsc")
        nc.gpsimd.memset(epsc[:], EPS)
        zconst = const.tile([P, 1], F32, tag="zconst")
        nc.gpsimd.memset(zconst[:], 0.0)
        nc.const_aps.aps[(F32, 0.0)] = zconst[:]

        # ---------- per-image persistent tiles ----------
        xb = [persist.tile([P, NCC, TOK], F16, tag=f"xb{i}", name=f"xb{i}")
              for i in range(IPC)]
        qk_sb = [persist.tile([P, 8, TOK], F16, tag=f"qk{i}", name=f"qk{i}")
                 for i in range(IPC)]
        vhat = [persist.tile([P, NJC, HEADS, 65], F16, tag=f"vh{i}", name=f"vh{i}")
                for i in range(IPC)]
        outT = [persist.tile([P, NCC, TOK], F16, tag=f"ot{i}", name=f"ot{i}")
                for i in range(IPC)]
        rsbc = [persist.tile([P, TOK], F16, tag=f"rs{i}", name=f"rs{i}")
                for i in range(IPC)]
        mu16 = [persist.tile([1, TOK], F16, tag=f"mu{i}", name=f"mu{i}")
                for i in range(IPC)]

        for i in range(IPC):
            nc.vector.memset(vhat[i][:, :, :, 64:65], 1.0)

        # x loads for both images up front (swdge queues, off engines)
        for img in range(IPC):
            for ci in range(NCC):
                nc.gpsimd.dma_start(xb[img][:, ci, :],
                                    x_d[img, ci * P:(ci + 1) * P, :])

        # ================= LN stats (prologue, both images) ============
        def stats(img):
            sq = []
            for ci in range(NCC):
                xsq = accp.tile([P, TOK], F16, tag="acc", name=f"xsq{img}_{ci}")
                nc.vector.tensor_tensor(xsq[:], xb[img][:, ci, :],
                                        xb[img][:, ci, :], OP.mult)
                sq.append(xsq)
            sc1 = small.tile([1, TOK], F32, tag="sc1", name=f"sc1{img}")
            for hf in range(2):
                sl = slice(hf * 512, (hf + 1) * 512)
                st = proP.tile([P, 512], F32, tag="pro", name=f"st{img}_{hf}")
                for ci in range(NCC):
                    nc.tensor.matmul(st[0:1, :], lhsT=ones128[:],
                                     rhs=xb[img][:, ci, sl],
                                     start=(ci == 0), stop=(ci == NCC - 1))
                    nc.tensor.matmul(st[32:33, :], lhsT=ones128[:],
                                     rhs=sq[ci][:, sl],
                                     start=(ci == 0), stop=(ci == NCC - 1))
                nc.vector.tensor_scalar(mu16[img][0:1, sl], st[0:1, :],
                                        1.0 / C, None, OP.mult)
                nc.vector.tensor_tensor(sc1[0:1, sl], mu16[img][0:1, sl],
                                        mu16[img][0:1, sl], OP.mult)
                nc.vector.scalar_tensor_tensor(
                    sc1[0:1, sl], st[32:33, :], 1.0 / C, sc1[0:1, sl],
                    OP.mult, OP.subtract)
            # rstd = exp(-0.5 * ln(var + eps)); stays in the exp/ln table
            nc.scalar.activation(sc1[:], sc1[:], AF.Ln, bias=epsc[0:1, :])
            rs16 = small.tile([1, TOK], F16, tag="rs16", name=f"rs16{img}")
            nc.scalar.activation(rs16[:], sc1[:], AF.Exp, scale=-0.5)
            bc = proP.tile([P, TOK], F32, tag="pro", name=f"bc{img}")
            for hf in range(2):
                sl = slice(hf * 512, (hf + 1) * 512)
                nc.tensor.matmul(bc[:, sl], lhsT=onesrow[:],
                                 rhs=rs16[0:1, sl], start=True, stop=True)
            nc.scalar.activation(rsbc[img][:], bc[:], AF.Copy)

        stats(0)
        stats(1)

        # ================= stage 1 generator =================
        def s1(img):
            for oc_i, oc in enumerate(OC_ORDER):
                psq = psqP.tile([P, TOK], F32, tag="psq", name=f"psq{img}_{oc}")
                for hf in range(2):
                    sl = slice(hf * 512, (hf + 1) * 512)
                    for ci in range(NCC):
                        nc.tensor.matmul(
                            psq[:, sl],
                            lhsT=wqkvT[:, ci, oc * P:(oc + 1) * P],
                            rhs=xb[img][:, ci, sl],
                            start=(ci == 0), stop=False)
                    nc.tensor.matmul(
                        psq[:, sl],
                        lhsT=negwsum[0:1, oc * P:(oc + 1) * P],
                        rhs=mu16[img][0:1, sl],
                        start=False, stop=True)
                yield ("mm", img, oc_i)

                qkvp = qp.tile([P, PTOK], F16, tag="qkvp", name=f"qv{img}_{oc}")
                nc.gpsimd.memset(qkvp[:, 0:34], 0.0)
                edge = qkvp[:, 33:33 + 33 * PW].rearrange(
                    "p (r c) -> p r c", c=PW)[:, :, 0:2]
                nc.gpsimd.memset(edge, 0.0)
                nc.gpsimd.memset(qkvp[:, 33 * PW:PTOK], 0.0)
                # fused evacuation * rstd into the padded interior
                nc.vector.tensor_tensor(
                    _pad_view(qkvp, PW + 1, S),
                    psq[:].rearrange("p (x y) -> p x y", y=S),
                    rsbc[img][:].rearrange("p (x y) -> p x y", y=S),
                    OP.mult)
                yield ("evac", img, oc_i)

                # PE taps accumulate into the (reused) psq PSUM tile
                for ti, (dx, dy) in enumerate(PE_TAPS):
                    for hf in range(2):
                        rhs = _pad_view(qkvp, _tap_off(dx, dy) + hf * 16 * PW, 16)
                        nc.tensor.matmul(
                            psq[:, hf * 512:(hf + 1) * 512],
                            lhsT=dwdiag[:, oc, ti, :],
                            rhs=rhs,
                            start=(ti == 0), stop=(ti == len(PE_TAPS) - 1))
                yield ("taps", img, oc_i)

                # scalar taps -> merged on gpsimd
                sa = []
                for si, (dx, dy) in enumerate(SC_TAPS):
                    a = accp.tile([P, TOK], F16, tag="acc",
                                  name=f"sa{img}_{oc}_{si}")
                    nc.scalar.activation(
                        a[:].rearrange("p (x y) -> p x y", y=S),
                        _pad_view(qkvp, _tap_off(dx, dy), S),
                        AF.Copy,
                        scale=dwW[:, oc, _tidx((dx, dy)):_tidx((dx, dy)) + 1])
                    sa.append(a)
                gm = accp.tile([P, TOK], F16, tag="acc", name=f"gm{img}_{oc}")
                nc.gpsimd.tensor_tensor(gm[:], sa[0][:], sa[1][:], OP.add)
                yield ("sc", img, oc_i)

                # DVE tap fused with the tap-PSUM evacuation
                dx, dy = DV_TAP
                c1 = accp.tile([P, TOK], F16, tag="acc", name=f"c1{img}_{oc}")
                nc.vector.scalar_tensor_tensor(
                    c1[:].rearrange("p (x y) -> p x y", y=S),
                    _pad_view(qkvp, _tap_off(dx, dy), S),
                    dwW[:, oc, _tidx((dx, dy)):_tidx((dx, dy)) + 1],
                    psq[:].rearrange("p (x y) -> p x y", y=S),
                    OP.mult, OP.add)
                if oc < 8:
                    dest = qk_sb[img][:, oc, :]
                    vtmp = None
                else:
                    vtmp = vtp.tile([P, TOK], F16, tag="vtmp",
                                    name=f"vt{img}_{oc}")
                    dest = vtmp[:]
                nc.vector.tensor_tensor(dest, c1[:], gm[:], OP.add)
                yield ("chain", img, oc_i)

                if oc >= 8:
                    pr = oc - 8
                    for jc in range(NJC):
                        tt = ttp.tile([P, P], F16, tag="tt",
                                      name=f"tt{img}_{oc}_{jc}")
                        nc.sync.dma_start(
                            tt[:], vtmp[:, jc * P:(jc + 1) * P], transpose=True)
                        nc.gpsimd.tensor_copy(
                            out=vhat[img][:, jc, 2 * pr:2 * pr + 2, 0:64],
                            in_=tt[:].rearrange("p (h d) -> p h d", h=2))
                        if jc == 3:
                            yield ("vt", img, oc_i)
                yield ("ocdone", img, oc_i)

        # ================= stage 2 generator =================
        def s2(img, simP, avP):
            steps = [(h, jc) for h in range(HEADS) for jc in range(NJC)]
            ebq = {}
            avh = [None, None]

            def load_eb(t):
                h, jc = steps[t]
                eb = ep.tile([P, TOK], F16, tag="eb", name=f"eb{img}_{h}_{jc}")
                eng = nc.sync if t % 2 == 0 else nc.gpsimd
                eng.dma_start(eb[:], ebt_d[h, jc])
                ebq[t] = eb

            def av_flush(prev):
                h, jc, E = prev
                oc_q = h // 2
                r0 = (h % 2) * 64
                if jc == 0:
                    avh[0] = avP.tile([65, TOK], F32, tag="av",
                                      name=f"av{img}_{h}")
                av = avh[0]
                for hf in range(2):
                    sl = slice(hf * 512, (hf + 1) * 512)
                    nc.tensor.matmul(av[:, sl],
                                     lhsT=vhat[img][:, jc, h, :],
                                     rhs=E[:, sl],
                                     start=(jc == 0), stop=(jc == NJC - 1))
                if jc == NJC - 1:
                    # head output -> outT; denominator row -> dn pair tile
                    if h % 2 == 0:
                        nc.vector.tensor_copy(out=outT[img][r0:r0 + 64, oc_q, :],
                                              in_=av[0:64, :])
                    else:
                        nc.scalar.activation(outT[img][r0:r0 + 64, oc_q, :],
                                             av[0:64, :], AF.Copy)
                    # per-head denominators live along the free dim (engine
                    # partition bases are restricted to 0/32/64)
                    if h % 2 == 0:
                        avh[1] = rcp.tile([1, 2, TOK], F32, tag="dnp",
                                          name=f"dnp{img}_{h // 2}")
                    nc.scalar.activation(avh[1][0:1, h % 2, :],
                                         av[64:65, :], AF.Copy)
                    if h % 2 == 1:
                        pr = h // 2
                        dnp = avh[1]
                        rc32 = rcp.tile([1, 2, TOK], F32, tag="rc32",
                                        name=f"rc{img}_{pr}")
                        nc.vector.reciprocal_approx_fast(out=rc32[:], in_=dnp[:])
                        rc16 = rcp.tile([1, 2, TOK], F16, tag="rc16",
                                        name=f"rc16{img}_{pr}")
                        nc.vector.tensor_copy(out=rc16[:], in_=rc32[:])
                        ps_bc = simP.tile([P, TOK], F32, tag="sim",
                                          name=f"nb{img}_{pr}")
                        for hf in range(2):
                            sl = slice(hf * 512, (hf + 1) * 512)
                            nc.tensor.matmul(ps_bc[:, sl], lhsT=selA[:],
                                             rhs=rc16[0:1, 0, sl],
                                             start=True, stop=False)
                            nc.tensor.matmul(ps_bc[:, sl], lhsT=selB[:],
                                             rhs=rc16[0:1, 1, sl],
                                             start=False, stop=True)
                        rb = Ep.tile([P, TOK], F16, tag="ee",
                                     name=f"rb{img}_{pr}")
                        nc.vector.tensor_copy(out=rb[:], in_=ps_bc[:])
                        nc.vector.tensor_tensor(outT[img][:, pr, :],
                                                outT[img][:, pr, :], rb[:],
                                                OP.mult)

            for t0 in range(4):
                load_eb(t0)
            prev = None
            for t, (h, jc) in enumerate(steps):
                if t + 4 < len(steps):
                    load_eb(t + 4)
                eb = ebq.pop(t)
                oc_q = h // 2
                r0 = (h % 2) * 64
                ps_sim = simP.tile([P, TOK], F32, tag="sim",
                                   name=f"sim{img}_{h}_{jc}")
                lhsT = qk_sb[img][r0:r0 + 64, 4 + oc_q, jc * P:(jc + 1) * P]
                for hf in range(2):
                    sl = slice(hf * 512, (hf + 1) * 512)
                    nc.tensor.matmul(ps_sim[:, sl], lhsT=lhsT,
                                     rhs=qk_sb[img][r0:r0 + 64, oc_q, sl],
                                     start=True, stop=True)
                E = Ep.tile([P, TOK], F16, tag="ee", name=f"ee{img}_{h}_{jc}")
                nc.scalar.activation(E[:], ps_sim[:], AF.Exp)
                nc.vector.tensor_tensor(E[:], E[:], eb[:], OP.mult)
                yield ("sim", img, t)
                if prev is not None:
                    av_flush(prev)
                prev = (h, jc, E)
                yield ("step", img, t)
            av_flush(prev)
            yield ("fin", img, -1)

        # ================= stage 3 generator (out projection) =========
        def s3(img):
            for oc4 in range(NCC):
                ps_o = psqP.tile([P, TOK], F32, tag="psq", name=f"pso{img}_{oc4}")
                for hf in range(2):
                    sl = slice(hf * 512, (hf + 1) * 512)
                    for kc in range(NCC):
                        nc.tensor.matmul(
                            ps_o[:, sl],
                            lhsT=woutT[:, kc, oc4 * P:(oc4 + 1) * P],
                            rhs=outT[img][:, kc, sl],
                            start=(kc == 0), stop=(kc == NCC - 1))
                of = ofp.tile([P, TOK], F16, tag="of", name=f"of{img}_{oc4}")
                if oc4 % 2 == 0:
                    nc.scalar.activation(of[:], ps_o[:], AF.Copy)
                else:
                    nc.vector.tensor_copy(out=of[:], in_=ps_o[:])
                nc.gpsimd.dma_start(out_d[img, oc4 * P:(oc4 + 1) * P, :], of[:])
                yield ("s3", img, oc4)

        # ================= the weave =================
        # psqP has a single buffer, so its users (s1 oc-steps of either
        # image, s3 oc4-steps) must run one oc-step at a time to
        # completion; interleaving two of them at sub-step granularity
        # queues one's matmuls ahead of the other's taps -> deadlock.
        g1 = {0: s1(0), 1: s1(1)}

        def ocstep_iter(gen):
            while True:
                v = next(gen, None)
                if v is None or v[0] in ("ocdone", "s3"):
                    return
                yield

        # prologue: image-0 triple 0 (q0,k0,v0) unblocks the attention;
        # one img1 oc-step keeps PE fed under img0's scalar/DVE chain.
        for _ in range(3):
            for _ in ocstep_iter(g1[0]):
                pass
        for _ in ocstep_iter(g1[1]):
            pass

        pro_ctx.close()
        simP = ctx.enter_context(tc.tile_pool(name="simP", bufs=2, space="PSUM"))
        avP = ctx.enter_context(tc.tile_pool(name="avP", bufs=1, space="PSUM"))

        g2 = {0: s2(0, simP, avP), 1: s2(1, simP, avP)}
        g3 = {0: s3(0), 1: s3(1)}

        # (start_slot, kind, img): remaining psqP oc-steps, paced so each
        # image's qkv triples complete before the attention needs them.
        steps_plan = [
            (0, "s1", 0), (4, "s1", 0), (8, "s1", 0), (12, "s1", 1),
            (16, "s1", 0), (20, "s1", 0), (24, "s1", 0), (28, "s1", 1),
            (32, "s1", 0), (36, "s1", 0), (40, "s1", 0), (44, "s1", 1),
            (50, "s1", 1), (56, "s1", 1), (62, "s1", 1), (68, "s1", 1),
            (73, "s3", 0), (74, "s1", 1), (79, "s3", 0), (80, "s1", 1),
            (85, "s3", 0), (86, "s1", 1), (91, "s3", 0), (92, "s1", 1),
        ]
        pi = 0
        cur = None
        NSLOT = 130

        def pump():
            nonlocal cur
            if cur is not None and next(cur, "END") == "END":
                cur = None

        for gslot in range(NSLOT):
            i2 = 0 if gslot < NSLOT // 2 else 1
            if cur is None and pi < len(steps_plan) and \
                    steps_plan[pi][0] <= gslot:
                kind, who = steps_plan[pi][1], steps_plan[pi][2]
                cur = ocstep_iter(g1[who] if kind == "s1" else g3[who])
                pi += 1
            pump()
            next(g2[i2], None)
            pump()
            next(g2[i2], None)

        # epilogue: drain stragglers, then image-1 out-projection
        while cur is not None or pi < len(steps_plan):
            if cur is None:
                kind, who = steps_plan[pi][1], steps_plan[pi][2]
                cur = ocstep_iter(g1[who] if kind == "s1" else g3[who])
                pi += 1
            pump()
        for _ in g2[0]:
            pass
        for _ in g2[1]:
            pass
        for _ in g3[0]:
            pass
        for _ in g3[1]:
            pass

    return nc


# ------------------------- host side -------------------------

def _rel_pos_indices(size):
    ar = np.arange(size)
    pos = np.stack(np.meshgrid(ar, ar, indexing="ij"), axis=-1).reshape(-1, 2)
    rel = pos[:, None, :] - pos[None, :, :] + size - 1
    return rel[..., 0] * (2 * size - 1) + rel[..., 1]


_NC_CACHE = None


def _get_nc():
    global _NC_CACHE
    if _NC_CACHE is None:
        _NC_CACHE = build_nc()
        _NC_CACHE.finalize()
    return _NC_CACHE


def kernel(x, gamma, w_qkv, dw_w_q, dw_b_q, dw_w_k, dw_b_k, dw_w_v, dw_b_v,
           w_out, pos_emb):
    x = np.asarray(x, np.float32).reshape(B, C, TOK)
    gamma_c = np.asarray(gamma, np.float32).reshape(C)
    w_qkv = np.asarray(w_qkv, np.float32)
    w_out = np.asarray(w_out, np.float32)
    pos_emb = np.asarray(pos_emb, np.float32)

    # fold gamma into qkv weights; transpose to (c, o); chunk for SBUF layout
    w_eff = w_qkv * gamma_c[None, :]
    wqkvT = np.ascontiguousarray(
        w_eff.T.reshape(NCC, P, O3).transpose(1, 0, 2)).astype(np.float16)
    negwsum = (-w_eff.sum(axis=1))[None, :].astype(np.float16)
    woutT = np.ascontiguousarray(
        w_out.T.reshape(NCC, P, INNER).transpose(1, 0, 2)).astype(np.float16)

    # depthwise taps: (o, 9) in canonical TAPS order, q taps folded with scale
    dww = np.concatenate([
        np.asarray(dw_w_q, np.float32).reshape(INNER, 9) * SCALE,
        np.asarray(dw_w_k, np.float32).reshape(INNER, 9),
        np.asarray(dw_w_v, np.float32).reshape(INNER, 9)], axis=0)
    dwb = np.concatenate([
        np.asarray(dw_b_q, np.float32) * SCALE,
        np.asarray(dw_b_k, np.float32),
        np.asarray(dw_b_v, np.float32)], axis=0)
    assert np.all(dwb == 0.0), "nonzero dwconv bias not supported by this kernel"
    dwW = np.ascontiguousarray(
        dww.reshape(NOC, P, 9).transpose(1, 0, 2)).astype(np.float32)

    dwdiag = np.zeros((P, NOC, len(PE_TAPS), P), np.float32)
    for oc in range(NOC):
        for ti, tap in enumerate(PE_TAPS):
            col = TAPS.index(tap)
            for p in range(P):
                dwdiag[p, oc, ti, p] = dww[oc * P + p, col]
    dwdiag = dwdiag.astype(np.float16)

    # exp of transposed relative-position bias: ebt[h, jc, j_in_chunk, i]
    idx = _rel_pos_indices(S)                       # (TOK, TOK)
    bias = pos_emb[idx]                             # (i, j, h)
    ebt = np.exp(bias.transpose(2, 1, 0))           # (h, j, i)
    ebt = np.ascontiguousarray(
        ebt.reshape(HEADS, NJC, P, TOK)).astype(np.float16)

    selpair = np.zeros((2, P), np.float16)
    selpair[0, :64] = 1.0
    selpair[1, 64:] = 1.0

    shared = dict(wqkvT=wqkvT, negwsum=negwsum, woutT=woutT, dwW=dwW,
                  dwdiag=dwdiag, ebt=ebt, selpair=selpair)
    in_maps = [dict(x=np.ascontiguousarray(x[i * IPC:(i + 1) * IPC]), **shared)
               for i in range(NCORES)]

    global last_in_maps
    last_in_maps = in_maps
    res = run_bass_kernel_spmd(_get_nc(), in_maps, list(range(NCORES)))
    out = np.concatenate([r["out"] for r in res.results], axis=0)
    return out.reshape(B, C, S, S).astype(np.float32)
